# revision 15
# baseline (speedup 1.0000x reference)
"""Expert-parallel MoE (top-2, E=8) for one Trainium2 chip (8 NeuronCores).

Contract: kernel(**inputs) takes the FULL unsharded inputs
  x  [4, 2048, 1024] f32,  Wr [1024, 8] f32,
  W1 [8, 1024, 2730] f32,  W2 [8, 2730, 1024] f32,  W3 [8, 1024, 2730] f32
and returns the FULL output [4, 2048, 1024] f32.

Sharding strategy (expert-parallel with H-split load balancing):
  - The tiny router (softmax + top-2 over 8 experts) runs on host in fp32.
  - Each expert's FFN is split along the hidden dim H=2730 into two
    1408-wide half-banks (half0 = h[0:1408], half1 = h[1408:2730] zero-
    padded to 1408).  A half-bank's output is a partial y summed on host.
  - That yields 16 (expert, half) jobs sized by the expert's token count.
    The 8 jobs of the 4 hottest experts go to slot A (one per core), the
    8 jobs of the 4 coldest to slot B.  Every core therefore processes
    Ca (= max hot count) + Cb (= max cold count) columns instead of
    2*max_count -- the SPMD padding waste drops from ~4.2% to ~1.8%.
  - Each core computes, per slot, the half-FFN out of SBUF-resident
    fp16 weights with fp32 PSUM accumulation:
        Y^T = W2h^T @ (silu(W1h^T X^T) * (W3h^T X^T))
  - Host combine: out[tok] = sum_k gate[tok,k] * (Y_half0 + Y_half1).

All device inputs are pre-tiled on host into the exact SBUF layout
(partition-major, h-tile-major) so every DMA moves >=4KB contiguous
runs per partition -- small-run strided DMAs are descriptor-limited to
~60-140 GB/s, which starves phase A of its weight stream at startup.
"""

import copy
import json
from contextlib import ExitStack

import numpy as np

# ---------------------------------------------------------------------------
# Walrus workaround: the neuronxcc walrus in this environment supports only
# ONE sync wait per instruction, while the Tile framework emits a final Drain
# carrying several.  Rewrite the serialized BIR: hoist extra waits into
# wait-only EventSemaphore instructions placed immediately before, on the
# same engine (the sequencer blocks on them in program order, so the
# semantics are unchanged).
# ---------------------------------------------------------------------------


def _split_multiwait_bir(bir_json):
    d = json.loads(bir_json)
    changed = False
    multi_update = []
    for fn in d.get("functions", []):

        def walk(block):
            nonlocal changed
            il = block.get("instructions")
            if il:
                new = []
                blk_changed = False
                for i in il:
                    si = i.get("sync_info") or {}
                    ws = si.get("on_wait") or []
                    if len(ws) > 1:
                        for j, w in enumerate(ws[:-1]):
                            new.append(
                                {
                                    "debug": i.get("debug"),
                                    "engine": i["engine"],
                                    "ins": [],
                                    "outs": [],
                                    "name": f"{i['name']}_xw{j}",
                                    "opcode": "EventSemaphore",
                                    "sync_info": {"on_update": [], "on_wait": [w]},
                                }
                            )
                        i = copy.deepcopy(i)
                        i["sync_info"]["on_wait"] = [ws[-1]]
                        blk_changed = True
                    us = (i.get("sync_info") or {}).get("on_update") or []
                    if len(us) > 1:
                        multi_update.append((i.get("name"), i.get("opcode")))
                    new.append(i)
                if blk_changed:
                    block["instructions"] = new
                    changed = True
            for b in block.get("blocks", []) or []:
                walk(b)

        walk(fn)

        # Trim the post-drain barrier/sem-clear tail of the TileContext end
        # block (~5-10 us of EVSEM butterfly).  The Drain already guarantees
        # all output DMAs completed; sems are re-initialized by the preamble
        # on the next execution (verified by back-to-back runs).
        def trim(block):
            nonlocal changed
            il = block.get("instructions")
            if il and block.get("name", "").endswith("_end"):
                last_drain = None
                for idx, i in enumerate(il):
                    if i.get("opcode") == "Drain" and i.get("engine") == "SP":
                        last_drain = idx
                        break
                if last_drain is not None and last_drain + 1 < len(il):
                    block["instructions"] = il[: last_drain + 1]
                    changed = True
            for b in block.get("blocks", []) or []:
                trim(b)

        trim(fn)
    if multi_update:
        raise RuntimeError(f"multi-update instructions unsupported: {multi_update[:5]}")
    if not changed:
        return bir_json
    return json.dumps(d).encode()


_patched = False


def _install_bir_patch():
    global _patched
    if _patched:
        return
    import concourse.bass2jax as b2j

    orig = b2j.compile_bir_kernel

    def patched(bir_json, tmpdir, neff_name="file.neff"):
        return orig(_split_multiwait_bir(bir_json), tmpdir, neff_name)

    b2j.compile_bir_kernel = patched
    _patched = True


_install_bir_patch()

import concourse.bass as bass
import concourse.mybir as mybir
import concourse.tile as tile
from concourse.bass_utils import run_bass_kernel_spmd

D = 1024
E = 8
TOP_K = 2
H = 2730
HB = 1408  # half-bank width (11 * 128); half1 holds 1322 real + 86 zero cols
DT = mybir.dt.float16
NP_DT = np.float16
D_TILES = D // 128  # 8
H_TILES = HB // 128  # 11
WCOLS = H_TILES * D  # flat per-partition weight columns (11 * 1024)


LEAD_A = 256  # small first block: its x DMA + first weight chunks gate start


def _plans(Ca, Cb):
    return _plan_blocks(Ca, lead=LEAD_A), _plan_blocks(Cb)


def _plan_blocks(C, lead=0):
    """Column blocks of <=512.  `lead` carves a small first block (its x
    DMA + first weight chunks gate the kernel start, so small = early);
    any remainder block is placed second-ish and the last block is full
    (dense tail)."""
    sizes = []
    if lead and C > lead:
        sizes.append(lead)
        C -= lead
    n_full, rem = divmod(C, 512)
    tail = [512] * n_full
    if rem:
        tail.insert(min(1, len(tail)), rem)
    sizes += tail
    blocks = []
    off = 0
    for s in sizes:
        blocks.append((off, s))
        off += s
    return blocks


def _build_nc(Ca, Cb):
    CT = Ca + Cb
    nc = bass.Bass()
    f32 = mybir.dt.float32

    # All inputs pre-tiled to SBUF layout: weights [128, 11*1024] with
    # columns (h_tile, d_tile, 128) for w1/w3 and (h_tile, m) for w2;
    # x block-major [128, 8*CT] with block (off, TB) at flat cols
    # [8*off, 8*(off+TB)) laid out (d_tile, col).
    xt = nc.dram_tensor("xt", [128, D_TILES * CT], DT, kind="ExternalInput")
    wts = {
        n: nc.dram_tensor(n, [128, WCOLS], DT, kind="ExternalInput")
        for n in ("w1a", "w3a", "w2a", "w1b", "w3b", "w2b")
    }
    yt = nc.dram_tensor("yt", [D, CT], f32, kind="ExternalOutput")

    blocks_a, blocks_b = _plans(Ca, Cb)

    with tile.TileContext(nc) as tc, ExitStack() as ctx:
        wpool = ctx.enter_context(tc.tile_pool(name="w", bufs=1))
        xpool = ctx.enter_context(tc.tile_pool(name="x", bufs=2))
        hpool = ctx.enter_context(tc.tile_pool(name="h", bufs=2))
        spool = ctx.enter_context(tc.tile_pool(name="s", bufs=3))
        ypool = ctx.enter_context(tc.tile_pool(name="y", bufs=4))
        psA = ctx.enter_context(tc.tile_pool(name="psA", bufs=4, space="PSUM"))
        psY = ctx.enter_context(tc.tile_pool(name="psY", bufs=3, space="PSUM"))

        # Dependency-free warmup matmuls: keep the PE busy from body start
        # (~7.6us, after the fixed runtime preamble) until the first weight
        # chunk + x block land (~10.5us), so the HAM clock gate opens
        # (1.2 -> 2.4 GHz) before the first real matmul group.
        warm = ypool.tile([128, 256], DT, tag="warm")
        wps = psA.tile([128, 512], f32, tag="psA")
        for _ in range(16):
            nc.tensor.matmul(
                wps[:, :256], lhsT=warm[:, :128], rhs=warm[:, :256], start=True, stop=True
            )
        # warm is read uninitialized on purpose: the products land in a PSUM
        # tile that is never consumed, and skipping the memset removes the
        # DVE-preamble dependency so the PE warms from body start.
        nc.vector.memset(warm[:], 0.0)

        w_sb = {
            n: wpool.tile([128, WCOLS], DT, tag=n, name=f"{n}_sb")
            for n in ("w1a", "w3a", "w2a", "w1b", "w3b", "w2b")
        }

        # First x block (small, 512KB) on gpsimd so it lands with the first
        # weight chunks and the first matmul group unblocks early.
        TB0 = blocks_a[0][1]
        x_pre = xpool.tile([128, D_TILES * TB0], DT, tag="x")
        nc.gpsimd.dma_start(x_pre[:], xt[:, : D_TILES * TB0])

        # Weight streams (h-tile chunks; every chunk is contiguous in DRAM
        # and per-partition-contiguous in SBUF):
        #   sync   : w1a ramp, w2a, then all y stores
        #   scalar : w3a ramp, x block1, w1b/w3b/w2b
        #   gpsimd : x0, x blocks 2..n
        def wchunks(eng, name, tiles):
            h0 = 0
            for t in tiles:
                sl = slice(h0 * D, (h0 + t) * D)
                eng.dma_start(w_sb[name][:, sl], wts[name][:, sl])
                h0 += t
            assert h0 == H_TILES

        wchunks(nc.sync, "w1a", [1, 1, 1, 2, 3, 3])
        wchunks(nc.scalar, "w3a", [1, 1, 1, 2, 3, 3])
        wchunks(nc.sync, "w2a", [6, 5])

        x1_tile = None
        if len(blocks_a) > 1:
            off1, TB1 = blocks_a[1]
            x1_tile = xpool.tile([128, D_TILES * TB1], DT, tag="x", name="x1_sb")
            nc.scalar.dma_start(
                x1_tile[:], xt[:, D_TILES * off1 : D_TILES * (off1 + TB1)]
            )

        # Slot-B weights are needed only ~halfway through the run; their
        # dma_start dispatches are emitted BETWEEN early blocks so they sit
        # in the scalar (ACT) instruction stream after block 0/1's silu
        # activations instead of delaying the first one (each dispatch costs
        # ~650ns of serial ACT sequencer time).
        bweights = [("w1b", [6, 5]), ("w3b", [6, 5]), ("w2b", [6, 5])]

        def phase_a(bank, off, TB, x_sb):
            """H^T[:, block] = silu(W1^T X^T) * (W3^T X^T), fp16."""
            w1s, w3s = w_sb[f"w1{bank}"], w_sb[f"w3{bank}"]
            h_sb = hpool.tile([128, H_TILES * TB], DT, tag="h", name="h_sb")
            for h_i in range(H_TILES):
                ps1 = psA.tile([128, TB], f32, tag="psA", name="ps1")
                for d_i in range(D_TILES):
                    nc.tensor.matmul(
                        ps1,
                        lhsT=w1s[:, h_i * D + d_i * 128 : h_i * D + (d_i + 1) * 128],
                        rhs=x_sb[:, d_i * TB : (d_i + 1) * TB],
                        start=(d_i == 0),
                        stop=(d_i == D_TILES - 1),
                    )
                ps3 = psA.tile([128, TB], f32, tag="psA", name="ps3")
                for d_i in range(D_TILES):
                    nc.tensor.matmul(
                        ps3,
                        lhsT=w3s[:, h_i * D + d_i * 128 : h_i * D + (d_i + 1) * 128],
                        rhs=x_sb[:, d_i * TB : (d_i + 1) * TB],
                        start=(d_i == 0),
                        stop=(d_i == D_TILES - 1),
                    )
                sil = spool.tile([128, TB], f32, tag="sil", name="sil")
                nc.scalar.activation(sil, ps1, mybir.ActivationFunctionType.Silu)
                nc.vector.tensor_mul(h_sb[:, h_i * TB : (h_i + 1) * TB], sil, ps3)
            return h_sb

        def phase_b(bank, off, TB, base, h_sb):
            """Y^T[:, block] = W2^T @ H^T."""
            w2s = w_sb[f"w2{bank}"]
            for m_i in range(D_TILES):
                psy = psY.tile([128, TB], f32, tag="psY", name="psy")
                for h_i in range(H_TILES):
                    nc.tensor.matmul(
                        psy,
                        lhsT=w2s[:, h_i * D + m_i * 128 : h_i * D + (m_i + 1) * 128],
                        rhs=h_sb[:, h_i * TB : (h_i + 1) * TB],
                        start=(h_i == 0),
                        stop=(h_i == H_TILES - 1),
                    )
                y_sb = ypool.tile([128, TB], f32, tag="y", name="y_sb")
                nc.vector.tensor_copy(y_sb, psy)
                nc.sync.dma_start(
                    yt[m_i * 128 : (m_i + 1) * 128, base + off : base + off + TB],
                    y_sb,
                )

        # Software-pipelined schedule A0 A1 B0 A2 B1 ... : phase B trails
        # phase A by one block, relaxing the W2a deadline by a whole block
        # (phase B of the small lead block would otherwise outrun the
        # aggregate HBM stream) while the PE stays dense.
        all_blocks = [("a", off, TB, 0) for off, TB in blocks_a] + [
            ("b", off, TB, Ca) for off, TB in blocks_b
        ]
        pending = None
        for bi, (bank, off, TB, base) in enumerate(all_blocks):
            if bi >= 2 and bweights:
                n, tiles = bweights.pop(0)
                wchunks(nc.scalar, n, tiles)
            if bank == "b" and bweights:
                while bweights:
                    n, tiles = bweights.pop(0)
                    wchunks(nc.scalar, n, tiles)
            if bi == 0:
                x_sb = x_pre
            elif bi == 1:
                x_sb = x1_tile
            else:
                x_sb = xpool.tile([128, D_TILES * TB], DT, tag="x", name="x_sb")
                fo = D_TILES * (base + off)
                nc.gpsimd.dma_start(x_sb[:], xt[:, fo : fo + D_TILES * TB])
            h_sb = phase_a(bank, off, TB, x_sb)
            if pending is not None:
                phase_b(*pending)
            pending = (bank, off, TB, base, h_sb)
        phase_b(*pending)

    return nc


def _route(flat, Wr):
    N = flat.shape[0]
    logits = flat @ Wr
    m = logits.max(-1, keepdims=True)
    p = np.exp(logits - m)
    p /= p.sum(-1, keepdims=True)
    topi = np.argsort(-p, axis=-1)[:, :TOP_K]
    topv = np.take_along_axis(p, topi, -1)

    assign_tok = np.tile(np.arange(N), TOP_K)
    assign_exp = topi.T.ravel()
    order = np.argsort(assign_exp, kind="stable")
    counts = np.bincount(assign_exp, minlength=E)
    starts = np.zeros(E + 1, np.int64)
    starts[1:] = np.cumsum(counts)
    gate_flat = topv.T.ravel()
    return assign_tok, order, counts, starts, gate_flat


def _half_slice(half):
    return slice(0, HB) if half == 0 else slice(HB, H)


def _tile_w13(w):
    """W1/W3 half [1024, hb<=1408] -> [128, 11*1024] laid out
    (h_tile, d_tile, 128col) per partition."""
    o = np.zeros((D_TILES, 128, H_TILES, 128), NP_DT)
    hb = w.shape[1]
    o.reshape(D, HB)[:, :hb] = w
    return np.ascontiguousarray(o.transpose(1, 2, 0, 3).reshape(128, WCOLS))


def _tile_w2(w):
    """W2 half [hb<=1408, 1024] -> [128, 11*1024] laid out (h_tile, m)."""
    o = np.zeros((H_TILES, 128, D), NP_DT)
    o.reshape(HB, D)[: w.shape[0]] = w
    return np.ascontiguousarray(o.transpose(1, 0, 2).reshape(128, WCOLS))


_NC_CACHE = {}


def kernel(x, Wr, W1, W2, W3, _trace=False, _result=None):
    x = np.asarray(x)
    Wr = np.asarray(Wr, dtype=np.float32)
    W1 = np.asarray(W1)
    W2 = np.asarray(W2)
    W3 = np.asarray(W3)
    Bx, Tx, Dx = x.shape
    N = Bx * Tx
    flat = np.ascontiguousarray(x.reshape(N, Dx).astype(np.float32))

    assign_tok, order, counts, starts, gate_flat = _route(flat, Wr)

    # 16 (expert, half) jobs: the 4 hottest experts' two halves fill slot A
    # (one per core), the 4 coldest experts' halves fill slot B.
    by_count = np.argsort(-counts, kind="stable")
    hot, cold = by_count[:4], by_count[4:]
    Ca = max(128, int(counts[hot].max()))
    Cb = max(128, int(counts[cold].max()))
    jobs_a = [(int(e), half) for half in (0, 1) for e in hot]
    jobs_b = [(int(e), half) for half in (0, 1) for e in cold]

    blocks_a, blocks_b = _plans(Ca, Cb)

    flat16 = flat.astype(NP_DT)
    xe = {}
    tok_e = {}
    for e in range(E):
        idx = assign_tok[order[starts[e] : starts[e + 1]]]
        tok_e[e] = idx
        # [128, 8, cnt]: partition-major, d-tile, column
        xcols = flat16[idx].T.reshape(D_TILES, 128, -1).transpose(1, 0, 2)
        xe[e] = xcols

    def pack_x(xt, base_flat, Cs, blocks, xc):
        buf = np.zeros((128, D_TILES, Cs), NP_DT)
        buf[:, :, : xc.shape[2]] = xc
        for off, TB in blocks:
            seg = buf[:, :, off : off + TB].reshape(128, D_TILES * TB)
            fo = base_flat + D_TILES * off
            xt[:, fo : fo + D_TILES * TB] = seg

    W116 = W1.astype(NP_DT)
    W216 = W2.astype(NP_DT)
    W316 = W3.astype(NP_DT)

    in_maps = []
    for core in range(E):
        ea, ha = jobs_a[core]
        eb, hb = jobs_b[core]
        xt = np.zeros((128, D_TILES * (Ca + Cb)), NP_DT)
        pack_x(xt, 0, Ca, blocks_a, xe[ea])
        pack_x(xt, D_TILES * Ca, Cb, blocks_b, xe[eb])
        sa, sb = _half_slice(ha), _half_slice(hb)
        in_maps.append(
            {
                "xt": xt,
                "w1a": _tile_w13(W116[ea][:, sa]),
                "w3a": _tile_w13(W316[ea][:, sa]),
                "w2a": _tile_w2(W216[ea][sa, :]),
                "w1b": _tile_w13(W116[eb][:, sb]),
                "w3b": _tile_w13(W316[eb][:, sb]),
                "w2b": _tile_w2(W216[eb][sb, :]),
            }
        )

    key = (Ca, Cb)
    if key not in _NC_CACHE:
        _NC_CACHE[key] = _build_nc(Ca, Cb)
    nc = _NC_CACHE[key]

    res = run_bass_kernel_spmd(nc, in_maps, list(range(E)), trace=_trace)
    if _result is not None:
        _result.append(res)

    # Host combine: sum the two half-bank partials per expert, then apply
    # the top-2 gates.
    Yhalf = {}
    for core in range(E):
        yt = res.results[core]["yt"]
        ea, ha = jobs_a[core]
        eb, hb = jobs_b[core]
        Yhalf[(ea, ha)] = yt[:, : counts[ea]]
        Yhalf[(eb, hb)] = yt[:, Ca : Ca + counts[eb]]

    out = np.zeros((N, D), np.float32)
    for e in range(E):
        Ye = Yhalf[(e, 0)] + Yhalf[(e, 1)]  # [D, cnt]
        g = gate_flat[order[starts[e] : starts[e + 1]]]
        out[tok_e[e]] += g[:, None] * Ye.T
    return out.reshape(Bx, Tx, Dx).astype(x.dtype)
